# revision 22
# baseline (speedup 1.0000x reference)
"""MoE MLP (MegaBlocks-style, top-2 of 8 experts) on 8 Trainium2 NeuronCores.

Expert-parallel sharding: core e holds expert e's weights. The (tiny) router
runs on host and determines the sharding: tokens are gathered per expert
(the host-side analogue of the all-to-all dispatch), padded to a common
capacity CAP, and each core computes

    y_e = ( silu(x_e @ W1_e.T + b1_e) * (x_e @ W3_e.T + b3_e) ) @ W2_e.T

plus the w2 bias and the per-token router weight, entirely on device.
The host then scatter-adds each expert's rows back into the [T, H] output
(the unshard step; every token receives exactly its top-2 contributions).

Device layout: everything is kept feature-on-partitions / tokens-on-free:
  xt   = gathered tokens, transposed              (rhs of GEMM1)
  w13t = W13_e.T, gate/up pair-interleaved in 128-col chunks
  w2t  = W2_e.T
so both GEMMs contract over the partition axis with zero on-device
transposes. Matmul operands are fp16 (same 10-bit mantissa the PE's fast
fp32 path rounds to, half the HBM traffic and weight-load time);
accumulation is fp32 in PSUM and all bias/activation math is fp32.

DMA schedule: every input transfer goes on the sync HW-DGE queue in exact
consumption order (first k-tile of w13 slab 0, first k-slice of x, then the
rest), so the PE's first matmul starts ~4us in and the weight stream stays
ahead of compute with no mid-kernel stalls (which would also re-throttle
the PE clock via the HAM activity monitor). Output stores ride the
otherwise-idle GpSimd SW-DGE queue.
"""

import math
import os
from contextlib import ExitStack

import numpy as np

T, H, I, E = 1024, 1024, 1024, 8
TOP_K = 2
N_CORES = 8
P = 128
KT = H // P  # GEMM1 contraction k-tiles
NP = I // P  # gate/up chunk pairs
HC = H // P  # GEMM2 output h-chunks
IT = I // P  # GEMM2 contraction k-tiles

_NC_CACHE: dict[int, object] = {}
LAST_RESULTS = None


def _build(cap: int):
    """Build + compile the per-core Bass program for capacity `cap` tokens."""
    import concourse.mybir as mybir
    import concourse.tile as tile
    from concourse import bacc

    f32 = mybir.dt.float32
    f16 = mybir.dt.float16
    FT = mybir.ActivationFunctionType

    nc = bacc.Bacc("TRN2", target_bir_lowering=False, debug=False)

    # DRAM inputs, pre-tiled on host to the exact SBUF layouts
    # (partition-outermost so any slab range is per-partition contiguous).
    xt_d = nc.dram_tensor("xt", [P, KT, cap], f16, kind="ExternalInput").ap()
    # w13: one 256-col slab per (gate_j, up_j) pair -> [part, pair, kt, 256]
    w13_d = nc.dram_tensor("w13t", [P, NP, KT, 256], f16, kind="ExternalInput").ap()
    # w2: 256-col slabs over H -> [part, slab, it, 256]
    w2_d = nc.dram_tensor("w2t", [P, 4, IT, 256], f16, kind="ExternalInput").ap()
    # b13 [P,16] ++ b2 [P,8] ++ wv [P,cap], one transfer
    sm_d = nc.dram_tensor("smalls", [P, 24 + cap], f32, kind="ExternalInput").ap()
    y_d = nc.dram_tensor("y", [H, cap], f32, kind="ExternalOutput").ap()
    y_v = y_d.rearrange("(c p) t -> p c t", p=P)

    with tile.TileContext(nc) as tc, ExitStack() as ctx:
        consts = ctx.enter_context(tc.tile_pool(name="consts", bufs=1))
        actp = ctx.enter_context(tc.tile_pool(name="actp", bufs=2))
        temps = ctx.enter_context(tc.tile_pool(name="temps", bufs=4))
        psum = ctx.enter_context(tc.tile_pool(name="psum", bufs=2, space="PSUM"))
        psum2 = ctx.enter_context(tc.tile_pool(name="psum2", bufs=4, space="PSUM"))

        xts = consts.tile([P, KT, cap], f16)
        w13s = consts.tile([P, NP, KT, 256], f16)
        w2s = consts.tile([P, 4, IT, 256], f16)
        smalls = consts.tile([P, 24 + cap], f32)
        b13s = smalls[:, 0:16]
        b2s = smalls[:, 16:24]
        wvs = smalls[:, 24:]

        # Input DMA rides BOTH HW-DGE trigger queues (sync + scalar run
        # their transfers concurrently on the shared DMA engines): sync
        # streams the weights in consumption order, scalar streams the
        # tokens + smalls. Trigger issue costs ~0.7us on a sequencer, so
        # splitting across two engines removes the trigger-starvation that
        # otherwise paces the first ~1MB. Only the first k-tile transfers
        # are kept small to unblock the very first matmul.
        # j0's prefix streams as 2-k-tile chunks, slab chunks on sync and
        # token chunks on scalar, sized so the two queues stay byte-balanced
        # (neither races ahead into non-critical transfers while the other
        # still holds critical bytes) and trigger issue (~0.7us each) stays
        # just ahead of the data.
        for c in range(0, KT, 4):
            nc.sync.dma_start(w13s[:, 0, c : c + 4], w13_d[:, 0, c : c + 4])
            nc.scalar.dma_start(xts[:, c : c + 4], xt_d[:, c : c + 4])
        nc.sync.dma_start(w13s[:, 1], w13_d[:, 1])
        nc.sync.dma_start(w13s[:, 2], w13_d[:, 2])
        # smalls aren't consumed until the first activation (~main+13us), so
        # they queue behind s1/s2 to keep the critical slab bytes in front.
        nc.sync.dma_start(smalls[:], sm_d)
        nc.sync.dma_start(w13s[:, 3], w13_d[:, 3])
        nc.sync.dma_start(w13s[:, 4], w13_d[:, 4])
        # late transfers have slack: merge them to cut trigger + finalize cost
        nc.sync.dma_start(w13s[:, 5:8], w13_d[:, 5:8])
        nc.sync.dma_start(w2s[:, 0:2], w2_d[:, 0:2])
        nc.sync.dma_start(w2s[:, 2:4], w2_d[:, 2:4])

        # PE warm-up: the HAM clock gate holds the PE at 1.2GHz until it
        # has been busy ~3.4us. A short burst of throwaway matmuls on a
        # zeroed scratch tile (no DMA dependency, starts immediately)
        # overlaps that ramp with the initial DMA wait, so the real
        # stream runs at 2.4GHz almost from the start.
        warm_n = min(cap, 512)
        scratch = consts.tile([P, warm_n], f16)
        nc.gpsimd.memset(scratch[:], 0.0)
        wps = psum.tile([P, warm_n], f32, name="pg")
        for _ in range(10):
            nc.tensor.matmul(wps[:], scratch[:, 0:128], scratch[:], start=True, stop=True)

        for t0 in range(0, cap, 512):
            tw = min(512, cap - t0)
            tsl = slice(t0, t0 + tw)
            acts = actp.tile([P, IT, tw], f16)
            for j in range(NP):
                pg = psum.tile([P, tw], f32, name="pg")
                pu = psum.tile([P, tw], f32, name="pu")
                for kt in range(KT):
                    nc.tensor.matmul(
                        pg[:],
                        w13s[:, j, kt, 0:128],
                        xts[:, kt, tsl],
                        start=(kt == 0),
                        stop=(kt == KT - 1),
                    )
                for kt in range(KT):
                    nc.tensor.matmul(
                        pu[:],
                        w13s[:, j, kt, 128:256],
                        xts[:, kt, tsl],
                        start=(kt == 0),
                        stop=(kt == KT - 1),
                    )
                # Router weight wv is per-token (a GEMM2 column scale), and
                # GEMM2 is linear in columns: fold wv into the up branch
                # here so the GEMM2 epilogue is a single bias-activation.
                sg = temps.tile([P, tw], f32, name="sg")
                su = temps.tile([P, tw], f32, name="su")
                sw = temps.tile([P, tw], f32, name="sw")
                nc.scalar.activation(sg[:], pg[:], FT.Silu, bias=b13s[:, 2 * j : 2 * j + 1])
                nc.scalar.activation(
                    su[:], pu[:], FT.Identity, bias=b13s[:, 2 * j + 1 : 2 * j + 2]
                )
                nc.vector.tensor_mul(sw[:], su[:], wvs[:, tsl])
                nc.vector.tensor_mul(acts[:, j, :], sg[:], sw[:])
            for hc in range(HC):
                p2 = psum2.tile([P, tw], f32, name="p2")
                for it in range(IT):
                    nc.tensor.matmul(
                        p2[:],
                        w2s[:, hc // 2, it, (hc % 2) * 128 : (hc % 2) * 128 + 128],
                        acts[:, it, :],
                        start=(it == 0),
                        stop=(it == IT - 1),
                    )
                # wv is already folded in; the (wv-scaled) w2 bias is applied
                # by the host during the scatter-add, so the epilogue is just
                # the PSUM->SBUF evacuation (DMA cannot read PSUM directly).
                yb = temps.tile([P, tw], f32, name="yb")
                if hc < HC - 1:
                    nc.scalar.activation(yb[:], p2[:], FT.Identity)
                    nc.sync.dma_start(y_v[:, hc, tsl], yb[:])
                else:
                    # Last chunk is the tail latency: evacuate + store in two
                    # halves on independent engine/queue pairs.
                    h0 = tw // 2
                    s0_, s1_ = slice(t0, t0 + h0), slice(t0 + h0, t0 + tw)
                    nc.scalar.activation(yb[:, :h0], p2[:, :h0], FT.Identity)
                    nc.vector.tensor_copy(yb[:, h0:], p2[:, h0:])
                    nc.sync.dma_start(y_v[:, hc, s0_], yb[:, :h0])
                    nc.scalar.dma_start(y_v[:, hc, s1_], yb[:, h0:])

    nc.compile()
    return nc


def _get_nc(cap: int):
    nc = _NC_CACHE.get(cap)
    if nc is None:
        nc = _build(cap)
        _NC_CACHE[cap] = nc
    return nc


def _route(x, router_weight, router_bias):
    """Host router: top-2 expert ids + softmax weights per token (fp64 logits)."""
    logits = x.astype(np.float64) @ router_weight.astype(np.float64).T
    logits += router_bias.astype(np.float64)
    ar = np.arange(T)
    i1 = np.argmax(logits, axis=1)
    v1 = logits[ar, i1]
    l2 = logits.copy()
    l2[ar, i1] = -np.inf
    i2 = np.argmax(l2, axis=1)
    v2 = l2[ar, i2]
    e2 = np.exp(v2 - v1)
    g1 = (1.0 / (1.0 + e2)).astype(np.float32)
    g2 = (e2 / (1.0 + e2)).astype(np.float32)
    return i1, i2, g1, g2


def _tile_kxm(a):
    """[K, M] (K = contraction, multiple of 128) -> [P, K//P, M] SBUF layout."""
    k, m = a.shape
    return np.ascontiguousarray(a.reshape(k // P, P, m).transpose(1, 0, 2))


def kernel(x, router_weight, router_bias, w13, w13_bias, w2, w2_bias):
    from concourse.bass_utils import run_bass_kernel_spmd

    x = np.ascontiguousarray(np.asarray(x, dtype=np.float32))
    router_weight = np.asarray(router_weight, dtype=np.float32)
    router_bias = np.asarray(router_bias, dtype=np.float32)
    w13 = np.asarray(w13, dtype=np.float32)
    w13_bias = np.asarray(w13_bias, dtype=np.float32)
    w2 = np.asarray(w2, dtype=np.float32)
    w2_bias = np.asarray(w2_bias, dtype=np.float32)

    i1, i2, g1, g2 = _route(x, router_weight, router_bias)

    tok_idx, tok_w = [], []
    for e in range(E):
        m1 = i1 == e
        m2 = i2 == e
        idx_e = np.concatenate([np.nonzero(m1)[0], np.nonzero(m2)[0]])
        w_e = np.concatenate([g1[m1], g2[m2]]).astype(np.float32)
        tok_idx.append(idx_e)
        tok_w.append(w_e)

    counts = [len(ix) for ix in tok_idx]
    cap = max(64, int(math.ceil(max(counts) / 2.0)) * 2)

    in_maps = []
    for e in range(E):
        n = counts[e]
        xg = np.zeros((cap, H), np.float16)
        xg[:n] = x[tok_idx[e]]
        xt = _tile_kxm(np.ascontiguousarray(xg.T))  # [P, KT, cap]

        # pair-interleave gate/up rows in 128-row chunks
        w13_f16 = w13[e].astype(np.float16)  # [2I, H]
        wi = np.empty((2 * I, H), np.float16)
        wi.reshape(2 * NP, P, H)[0::2] = w13_f16[:I].reshape(NP, P, H)
        wi.reshape(2 * NP, P, H)[1::2] = w13_f16[I:].reshape(NP, P, H)
        # -> [H, 2I] -> [P, KT, 2I] -> slabs [NP, P, KT, 256]
        w13t = _tile_kxm(np.ascontiguousarray(wi.T))
        w13t = np.ascontiguousarray(w13t.reshape(P, KT, NP, 256).transpose(0, 2, 1, 3))

        bi = np.empty(2 * I, np.float32)
        bi.reshape(2 * NP, P)[0::2] = w13_bias[e, :I].reshape(NP, P)
        bi.reshape(2 * NP, P)[1::2] = w13_bias[e, I:].reshape(NP, P)
        b13 = np.ascontiguousarray(bi.reshape(2 * NP, P).T)  # [P, 16]

        w2t = _tile_kxm(np.ascontiguousarray(w2[e].T).astype(np.float16))
        w2t = np.ascontiguousarray(w2t.reshape(P, IT, 4, 256).transpose(0, 2, 1, 3))

        b2 = np.ascontiguousarray(w2_bias[e].reshape(HC, P).T)  # [P, 8]

        wv = np.zeros(cap, np.float32)
        wv[:n] = tok_w[e]
        wvb = np.broadcast_to(wv[None, :], (P, cap))
        smalls = np.ascontiguousarray(
            np.concatenate([b13, b2, wvb], axis=1, dtype=np.float32)
        )

        in_maps.append({"xt": xt, "w13t": w13t, "w2t": w2t, "smalls": smalls})

    nc = _get_nc(cap)
    res = run_bass_kernel_spmd(
        nc,
        in_maps,
        core_ids=list(range(N_CORES)),
        trace=os.environ.get("MOE_TRACE", "0") == "1",
    )
    global LAST_RESULTS
    LAST_RESULTS = res

    out = np.zeros((T, H), np.float32)
    for e in range(E):
        n = counts[e]
        if n:
            out[tok_idx[e]] += res.results[e]["y"][:, :n].T
            out[tok_idx[e]] += tok_w[e][:, None] * w2_bias[e][None, :]
    return out
